# revision 1
# baseline (speedup 1.0000x reference)
"""Trainium2 Bass kernel for nn_EnhancedEncoder (gnn_message_passing).

Data-parallel over the 1024 flattened groups: 128 groups per core on 8 cores.
All intermediates stay in SBUF. The KNN gather is reformulated as
block-diagonal adjacency matmuls; K_Norm reduces to count-weighted sums.
Cross-core reductions: one dummy warm-up AllReduce + BN3 stats + one merged
final round (global std of dx + fusion BN stats).

Position indexing per core: pos = t*128 + gp*32 + i  (t in [0,32), gp in
[0,4), i in [0,32)); group id g = 4*t + gp.  Channel-major tensors are
[ch_tile(128), pos(4096)]; feat tiles are [4*32 points, 384]; per-group
vectors are [*, g] with g = 4*t + gp.
"""
import sys
from contextlib import ExitStack

for _p in ("/opt/trn_rl_repo",):
    if _p not in sys.path:
        sys.path.insert(0, _p)

import numpy as np

NCORES = 8
G = 128            # groups per core
NPTS = 32          # points per group
NPOS = G * NPTS    # 4096 positions per core
C = 384            # encoder channels
K = 8              # knn group size
EPS = 1e-5
NTOT_POS = 1024 * NPTS          # global positions (BN1/BN3 denominator)
NTOT_DX = 1024 * NPTS * K * C   # global dx element count (std denominator)
NB = 1024                       # global batch of groups (BNf denominator)
BIG_NEG = -1e30
DEBUG = False

_BUILT = None


def _build(debug=False):
    import concourse.bacc as bacc
    import concourse.tile as tile
    from concourse import mybir

    f32 = mybir.dt.float32
    nc = bacc.Bacc("TRN2", target_bir_lowering=False, debug=False,
                   num_devices=NCORES)

    io = {}

    def din(name, shape):
        io[name] = nc.dram_tensor(name, shape, f32, kind="ExternalInput")

    din("pg", [G, NPTS, 3])
    din("pg_full", [1024, NPTS, 3])
    din("W1", [128, 3]); din("b1", [128])
    din("gamma1", [128]); din("beta1", [128])
    din("W2", [256, 128]); din("b2", [256])
    din("W3", [512, 512]); din("b3", [512])
    din("gamma3", [512]); din("beta3", [512])
    din("W4", [C, 512]); din("b4", [C])
    din("alpha", [C]); din("beta_aff", [C])
    din("Wf", [C, 2 * C]); din("bf", [C])
    din("gammaf", [C]); din("betaf", [C])
    io["out"] = nc.dram_tensor("out", [G, C], f32, kind="ExternalOutput")
    if debug:
        for nm, sh in [("dbg_f1h", [128, NPOS]), ("dbg_negkey", [128, 1024]),
                       ("dbg_A2", [128, 1024]), ("dbg_Kc", [128, 32]),
                       ("dbg_f3h0", [128, NPOS]), ("dbg_fg2", [128, C]),
                       ("dbg_U", [128, C]), ("dbg_mom", [128, 17]),
                       ("dbg_P", [128, C]), ("dbg_Q", [128, C]),
                       ("dbg_feat0", [128, C]), ("dbg_fg", [128, 256])]:
            io[nm] = nc.dram_tensor(nm, sh, f32, kind="ExternalOutput")

    with tile.TileContext(nc) as tc:
        _emit(nc, tc, tile, mybir, io, debug)
    nc.compile()
    return nc


def _emit(nc, tc, tile, mybir, io, debug):
    f32 = mybir.dt.float32
    f32r = mybir.dt.float32r
    Alu = mybir.AluOpType
    Act = mybir.ActivationFunctionType
    AX = mybir.AxisListType
    RG = [list(range(NCORES))]

    def mm(outap, lhsT, rhs, start, stop, rep=True):
        if rep:
            lhsT = lhsT.bitcast(f32r)
            rhs = rhs.bitcast(f32r)
        nc.tensor.matmul(outap, lhsT, rhs, start=start, stop=stop,
                         skip_group_check=True)

    ctx = ExitStack()
    per = ctx.enter_context(tc.tile_pool(name="per", bufs=1))
    ps_w = ctx.enter_context(tc.tile_pool(name="ps_w", bufs=2, space="PSUM"))
    ps_c = ctx.enter_context(tc.tile_pool(name="ps_c", bufs=2, space="PSUM"))
    dram = ctx.enter_context(tc.tile_pool(name="dram", bufs=1, space="DRAM"))

    # ---------------- constants ----------------
    ident = per.tile([128, 128], f32, name="ident")
    nc.gpsimd.memset(ident[:], 1.0)
    nc.gpsimd.affine_select(ident[:], ident[:], pattern=[[1, 128]],
                            compare_op=Alu.is_equal, fill=0.0, base=0,
                            channel_multiplier=-1)
    ones1x128 = per.tile([1, 128], f32, name="ones1x128")
    nc.gpsimd.memset(ones1x128[:], 1.0)
    nc.scalar.activation(ones1x128.bitcast(f32r), ones1x128[:], Act.Identity)
    ones128x1 = per.tile([128, 1], f32, name="ones128x1")
    nc.gpsimd.memset(ones128x1[:], 1.0)
    eps_col = per.tile([128, 1], f32, name="eps_col")
    nc.gpsimd.memset(eps_col[:], EPS)

    # ---------------- dummy collective (comm warm-up) ----------------
    warm_in = dram.tile([128, 1], f32, name="warm_in")
    warm_out = dram.tile([128, 1], f32, name="warm_out")
    nc.sync.dma_start(warm_in[:], ones128x1[:])
    nc.gpsimd.collective_compute("AllReduce", Alu.add, replica_groups=RG,
                                 ins=[warm_in.opt()], outs=[warm_out.opt()])

    # ---------------- load weights + vectors ----------------
    vstage_cm = tc.tile_pool(name="vstage", bufs=2)
    vstage = vstage_cm.__enter__()

    def vload(name, n):
        nt = n // 128
        st = vstage.tile([4, 128], f32, name=name + "_st", tag="vst")
        nc.sync.dma_start(st[:nt, :],
                          io[name].ap().rearrange("(a p) -> a p", p=128))
        pst = ps_c.tile([128, 4], f32, name=name + "_ps", tag="cps")
        nc.tensor.transpose(pst[:, :nt], st[:nt, :], ident[:nt, :nt])
        t = per.tile([128, nt], f32, name=name + "_sb")
        nc.vector.tensor_copy(t[:], pst[:, :nt])
        return t

    b1_sb = vload("b1", 128)
    gamma1_sb = vload("gamma1", 128)
    beta1_sb = vload("beta1", 128)
    b2_sb = vload("b2", 256)
    b3_sb = vload("b3", 512)
    gamma3_sb = vload("gamma3", 512)
    beta3_sb = vload("beta3", 512)
    b4_sb = vload("b4", C)
    bf_sb = vload("bf", C)
    gammaf_sb = vload("gammaf", C)
    betaf_sb = vload("betaf", C)
    betaaff_sb = vload("beta_aff", C)
    alpha_sb = vload("alpha", C)
    W1_sb = per.tile([128, 3], f32, name="W1_sb")
    nc.sync.dma_start(W1_sb[:], io["W1"].ap())
    vstage_cm.__exit__(None, None, None)
    W1T = per.tile([3, 128], f32, name="W1T")
    b4row = per.tile([1, C], f32, name="b4row")

    wts = {}
    with tc.tile_pool(name="wraw", bufs=1) as wraw:
        W1T0 = per.tile([3, 128], f32, name="W1T0")
        nc.sync.dma_start(W1T0[:], io["W1"].ap().rearrange("o c -> c o"))
        nc.scalar.activation(W1T.bitcast(f32r), W1T0[:], Act.Identity)
        b4row0 = per.tile([1, C], f32, name="b4row0")
        nc.sync.dma_start(b4row0[:], io["b4"].ap().unsqueeze(0))
        nc.scalar.activation(b4row.bitcast(f32r), b4row0[:], Act.Identity)
        for name, rows, cols in [("W2", 256, 128), ("W3", 512, 512),
                                 ("W4", C, 512), ("Wf", C, 2 * C)]:
            nr, kg = rows // 128, cols // 128
            raw = wraw.tile([128, nr * cols], f32, name=name + "_raw",
                            tag="wraw")
            nc.sync.dma_start(
                raw.rearrange("p (rt c) -> p rt c", rt=nr),
                io[name].ap().rearrange("(rt p) c -> p rt c", p=128))
            wt = per.tile([128, kg * nr * 128], f32, name=name + "T")
            for kc in range(kg):
                for mo in range(nr):
                    pst = ps_w.tile([128, 128], f32, name="wtps", tag="wtps")
                    nc.tensor.transpose(
                        pst[:],
                        raw[:, mo * cols + kc * 128:mo * cols + kc * 128 + 128],
                        ident[:])
                    dst = wt[:, (kc * nr + mo) * 128:
                             (kc * nr + mo) * 128 + 128].bitcast(f32r)
                    if (kc + mo) % 2 == 0:
                        nc.vector.tensor_copy(dst, pst[:])
                    else:
                        nc.scalar.activation(dst, pst[:], Act.Identity)
            wts[name] = wt
    W2T, W3T, W4T, WfT = wts["W2"], wts["W3"], wts["W4"], wts["Wf"]

    def wblk(wt, nr, kc, mo):
        return wt[:, (kc * nr + mo) * 128:(kc * nr + mo) * 128 + 128]

    def w4rhs(kc):  # pos-major rhs [128, 384] = blocks (kc, mo=0..2)
        return W4T[:, kc * 3 * 128:(kc * 3 + 3) * 128]

    # wbias = Wf[:, C:] @ beta_aff + bf   (channel-major [128, 3])
    wbias_ps = ps_c.tile([128, 3], f32, name="wbias_ps", tag="cps")
    for mo in range(3):
        for kc in range(3):
            mm(wbias_ps[:, mo:mo + 1], wblk(WfT, 3, 3 + kc, mo),
               betaaff_sb[:, kc:kc + 1], kc == 0, kc == 2, rep=False)
    wbias = per.tile([128, 3], f32, name="wbias")
    nc.vector.tensor_tensor(wbias[:], wbias_ps[:], bf_sb[:], op=Alu.add)

    def bn_scale_shift(var_ap, mu_ap, gam_ap, bet_ap, pref):
        std = per.tile([128, 1], f32, name=pref + "_std")
        nc.scalar.activation(std[:], var_ap, Act.Sqrt, bias=eps_col[:])
        rstd = per.tile([128, 1], f32, name=pref + "_rstd")
        nc.vector.reciprocal(rstd[:], std[:])
        sc = per.tile([128, 1], f32, name=pref + "_sc")
        nc.vector.tensor_tensor(sc[:], rstd[:], gam_ap, op=Alu.mult)
        sh = per.tile([128, 1], f32, name=pref + "_sh")
        nc.vector.tensor_tensor(sh[:], mu_ap, sc[:], op=Alu.mult)
        nc.vector.tensor_tensor(sh[:], bet_ap, sh[:], op=Alu.subtract)
        return sc, sh

    # ================ selection + BN1 moments (scoped pools) ============
    sel_b = tc.tile_pool(name="sel_b", bufs=1)
    sb = sel_b.__enter__()
    sel_a = tc.tile_pool(name="sel_a", bufs=1)
    sa = sel_a.__enter__()

    # ---- BN1 scale/shift from global input moments ----
    pgm = sa.tile([128, 768], f32, name="pgm")   # [128, (jj:8, i:32, c:3)]
    nc.gpsimd.dma_start(pgm[:], io["pg_full"].ap().rearrange(
        "(p jj) i c -> p (jj i c)", p=128).opt())
    mcols = sa.tile([128, 12], f32, name="mcols")
    pv = pgm.rearrange("p (j c) -> p j c", c=3)
    scr256 = sa.tile([128, 256], f32, name="scr256")
    for i in range(3):
        for j in range(3):
            nc.vector.scalar_tensor_tensor(
                scr256[:], pv[:, :, i], 1.0, pv[:, :, j],
                op0=Alu.mult, op1=Alu.mult,
                accum_out=mcols[:, 3 * i + j:3 * i + j + 1])
        nc.vector.tensor_reduce(mcols[:, 9 + i:10 + i], pv[:, :, i],
                                axis=AX.X, op=Alu.add)
    m12 = ps_c.tile([1, 12], f32, name="m12", tag="cps")
    mm(m12[:], ones128x1[:], mcols[:], True, True, rep=False)
    m12s = sa.tile([1, 12], f32, name="m12s")
    nc.scalar.activation(m12s[:], m12[:], Act.Identity, scale=1.0 / NTOT_POS)
    M2sb = sa.tile([3, 3], f32, name="M2sb")
    nc.sync.dma_start(M2sb[:],
                      m12s[:1, :9].rearrange("1 (i j) -> 1 i j", i=3))
    mu3 = sa.tile([3, 1], f32, name="mu3")
    nc.sync.dma_start(mu3[:], m12s[:1, 9:12])

    m1ps = ps_c.tile([128, 1], f32, name="m1ps", tag="cps")
    mm(m1ps[:], W1T0[:], mu3[:], True, True, rep=False)   # W1 @ mu_p
    mvec = per.tile([128, 1], f32, name="mvec")
    nc.vector.tensor_copy(mvec[:], m1ps[:])
    wmps = ps_c.tile([128, 3], f32, name="wmps", tag="cps")
    mm(wmps[:], W1T0[:], M2sb[:], True, True, rep=False)  # W1 @ M2
    e2raw = per.tile([128, 1], f32, name="e2raw")
    scr3 = per.tile([128, 3], f32, name="scr3")
    nc.vector.scalar_tensor_tensor(scr3[:], wmps[:], 1.0, W1_sb[:],
                                   op0=Alu.mult, op1=Alu.mult,
                                   accum_out=e2raw[:])
    mu1 = per.tile([128, 1], f32, name="mu1")
    nc.vector.tensor_tensor(mu1[:], mvec[:], b1_sb[:], op=Alu.add)
    t_a = per.tile([128, 1], f32, name="t_a")
    nc.vector.tensor_tensor(t_a[:], mvec[:], b1_sb[:], op=Alu.mult)
    e2 = per.tile([128, 1], f32, name="e2")
    nc.vector.scalar_tensor_tensor(e2[:], t_a[:], 2.0, e2raw[:],
                                   op0=Alu.mult, op1=Alu.add)
    nc.vector.tensor_tensor(t_a[:], b1_sb[:], b1_sb[:], op=Alu.mult)
    nc.vector.tensor_tensor(e2[:], e2[:], t_a[:], op=Alu.add)
    nc.vector.tensor_tensor(t_a[:], mu1[:], mu1[:], op=Alu.mult)
    var1 = per.tile([128, 1], f32, name="var1")
    nc.vector.tensor_tensor(var1[:], e2[:], t_a[:], op=Alu.subtract)
    sc1, sh1 = bn_scale_shift(var1[:], mu1[:], gamma1_sb[:], beta1_sb[:],
                              "bn1")
    # fold conv bias into the shift: f1h = relu(sc1*(W1x) + (sc1*b1 + sh1))
    sh1b = per.tile([128, 1], f32, name="sh1b")
    nc.vector.tensor_tensor(sh1b[:], sc1[:], b1_sb[:], op=Alu.mult)
    nc.vector.tensor_tensor(sh1b[:], sh1b[:], sh1[:], op=Alu.add)

    # ---- distance keys ----
    # pgA[32gp+i, (t, c)] = pg[4t+gp, i, c]
    pgA = sa.tile([128, 96], f32, name="pgA")
    pgr4 = io["pg"].ap().rearrange("(t gp) i c -> gp i t c", gp=4)
    for gp in range(4):
        nc.gpsimd.dma_start(
            pgA[32 * gp:32 * gp + 32].rearrange("i (t c) -> i t c", t=32).opt(),
            pgr4[gp].opt())
    # pgstage[gp, (t, m, c)] in DRAM, contiguous per gp
    pgstage = dram.tile([4, NPTS * 32 * 3], f32, name="pgstage")
    pgr5 = io["pg"].ap().rearrange("(t gp) m c -> gp t m c", gp=4)
    for gp in range(4):
        nc.gpsimd.dma_start(
            pgstage[gp:gp + 1].rearrange("1 (t m c) -> 1 t m c", t=32,
                                         m=32).opt(),
            pgr5[gp].opt())
    # pgB[32gp+n, (t, m, c)] = pg[4t+gp, m, c]   (replicated over n)
    pgB = sa.tile([128, 3072], f32, name="pgB")
    for gp in range(4):
        nc.gpsimd.dma_start(
            pgB[32 * gp:32 * gp + 32],
            pgstage[gp:gp + 1].broadcast_to([32, 3072]))
    # negkey[32gp+n, t*32+m] = sum_c (pgA[.,t,c] - 0.5*pgB_c)*pgB_c
    scr1 = sa.tile([128, 1024], f32, name="scr1")
    negkey = sb.tile([128, 1024], f32, name="negkey")
    for cdim in range(3):
        pgB_c = pgB.rearrange("p (t m c) -> p t m c", t=32, m=32)[:, :, :, cdim]
        pgA_c = pgA.rearrange("p (t c) -> p t c", c=3)[:, :, cdim] \
            .unsqueeze(2).broadcast_to([128, 32, 32])
        dst = scr1[:] if cdim > 0 else negkey[:]
        dstv = dst.rearrange("p (t m) -> p t m", t=32)
        nc.vector.scalar_tensor_tensor(dstv, pgB_c, -0.5, pgA_c,
                                       op0=Alu.mult, op1=Alu.add)
        nc.vector.tensor_tensor(dstv, dstv, pgB_c, op=Alu.mult)
        if cdim > 0:
            nc.vector.tensor_tensor(negkey[:], negkey[:], scr1[:],
                                    op=Alu.add)

    # x0[c, t*128+32gp+i] = pg[4t+gp, i, c]  via PE transpose of pgA2
    pgA2 = sa.tile([128, 96], f32, name="pgA2")  # [32gp+i, (c, t)]
    nc.vector.tensor_copy(
        pgA2.rearrange("p (c t) -> p c t", c=3),
        pgA.rearrange("p (t c) -> p c t", c=3))
    tps0 = ps_w.tile([128, 128], f32, name="wtps", tag="wtps")
    nc.tensor.transpose(tps0[0:96, :], pgA2[:], ident[:])
    T_sb = sa.tile([96, 128], f32, name="T_sb")
    nc.vector.tensor_copy(T_sb.bitcast(f32r), tps0[0:96, :])
    pgT_dram = dram.tile([3, NPOS], f32, name="pgT_dram")
    for cdim in range(3):
        nc.gpsimd.dma_start(
            pgT_dram[cdim:cdim + 1].rearrange("1 (t q) -> 1 t q", t=32).opt()
            .bitcast(f32r),
            T_sb[32 * cdim:32 * cdim + 32].bitcast(f32r))
    sel_a.__exit__(None, None, None)

    # ---- top-8 selection ----
    top8 = sb.tile([128, 8], f32, name="top8")
    repl = sb.tile([128, 1024], f32, name="repl", tag="repl")
    for t in range(32):
        nc.vector.max(top8[:], negkey[:, t * 32:(t + 1) * 32])
        nc.vector.match_replace(repl[:, t * 32:(t + 1) * 32], top8[:],
                                negkey[:, t * 32:(t + 1) * 32], BIG_NEG)
    A2 = sb.tile([128, 1024], f32, name="A2")
    nc.vector.tensor_scalar(A2[:], repl[:], BIG_NEG, None, op0=Alu.is_equal)
    if debug:
        nc.sync.dma_start(io["dbg_negkey"].ap(), negkey[:])
        nc.sync.dma_start(io["dbg_A2"].ap(), A2[:])

    # W_B build + counts from A2T
    A2T = sb.tile([128, 1024], f32, name="A2T", tag="repl")
    nc.vector.transpose(A2T[:], A2[:])
    # Kc[32gp+m, t] = K + sum_n A[n, m]
    Kc = per.tile([128, 32], f32, name="Kc")
    nc.vector.tensor_reduce(Kc[:],
                            A2T.rearrange("p (t n) -> p t n", t=32),
                            axis=AX.X, op=Alu.add)
    nc.vector.tensor_scalar(Kc[:], Kc[:], float(K), None, op0=Alu.add)
    if debug:
        nc.sync.dma_start(io["dbg_Kc"].ap(), Kc[:])

        # W_B[32gp+m, t*128+32gp+n] = A2T[32gp+m, t*32+n]  (block-diag lhsT)
    W_B = per.tile([128, NPOS], f32, name="W_B")
    nc.gpsimd.memset(W_B[:], 0.0)
    nc.vector.tensor_copy(W_B.bitcast(f32r), W_B[:])
    for gp in range(4):
        nc.vector.tensor_copy(
            W_B[32 * gp:32 * gp + 32].rearrange(
                "p (t q) -> p t q", t=32)[:, :, 32 * gp:32 * gp + 32]
            .bitcast(f32r),
            A2T[32 * gp:32 * gp + 32].rearrange("p (t n) -> p t n", t=32))
    sel_b.__exit__(None, None, None)

    # ================ conv1 / conv2 ================
    ps_mm_cm = tc.tile_pool(name="ps_mm", bufs=4, space="PSUM")
    ps_mm = ps_mm_cm.__enter__()
    act3_cm = tc.tile_pool(name="act3", bufs=1)
    act3 = act3_cm.__enter__()
    act1_cm = tc.tile_pool(name="act1", bufs=1)
    act1 = act1_cm.__enter__()

    x0 = act1.tile([3, NPOS], f32, name="x0")
    nc.gpsimd.dma_start(x0.bitcast(f32r), pgT_dram[:].bitcast(f32r))
    f1h = act1.tile([128, NPOS], f32, name="f1h")
    for nt in range(8):
        ps = ps_mm.tile([128, 512], f32, name="mmps", tag="mmps")
        mm(ps[:], W1T[:], x0[:, nt * 512:(nt + 1) * 512], True, True)
        nc.scalar.activation(f1h[:, nt * 512:(nt + 1) * 512].bitcast(f32r),
                             ps[:], Act.Relu, bias=sh1b[:], scale=sc1[:])
    if debug:
        nc.sync.dma_start(io["dbg_f1h"].ap(), f1h[:])

    f2 = [act3.tile([128, NPOS], f32, name=f"f2_{mo}") for mo in range(2)]
    for mo in range(2):
        for nt in range(8):
            ps = ps_mm.tile([128, 512], f32, name="mmps", tag="mmps")
            mm(ps[:], wblk(W2T, 2, 0, mo), f1h[:, nt * 512:(nt + 1) * 512],
               True, True)
            nc.scalar.activation(
                f2[mo][:, nt * 512:(nt + 1) * 512].bitcast(f32r), ps[:],
                Act.Identity, bias=b2_sb[:, mo:mo + 1])
    act1_cm.__exit__(None, None, None)
    fg = per.tile([128, 256], f32, name="fg")  # [128, (mo:2, g:128)]
    for mo in range(2):
        nc.vector.tensor_reduce(fg[:, mo * 128:(mo + 1) * 128]
                                .bitcast(f32r),
                                f2[mo].rearrange("p (g i) -> p g i", i=32),
                                axis=AX.X, op=Alu.max)
    if debug:
        nc.sync.dma_start(io["dbg_fg"].ap(), fg[:])

    # ================ conv3 + BN3 ================
    f3 = [per.tile([128, NPOS], f32, name=f"f3_{mo}") for mo in range(4)]
    stats3 = per.tile([128, 4 * 8 * 6], f32, name="stats3")
    for mo in range(4):
        for ntc in range(4):
            pss = [ps_mm.tile([128, 512], f32, name="mmps", tag="mmps")
                   for _ in range(2)]
            for kc in range(4):
                for j, nt in enumerate((2 * ntc, 2 * ntc + 1)):
                    if kc < 2:
                        rhs = fg[:, kc * 128 + nt * 16:
                                 kc * 128 + (nt + 1) * 16] \
                            .unsqueeze(2).broadcast_to([128, 16, 32])
                    else:
                        rhs = f2[kc - 2][:, nt * 512:(nt + 1) * 512]
                    mm(pss[j][:], wblk(W3T, 4, kc, mo), rhs, kc == 0,
                       kc == 3)
            for j, nt in enumerate((2 * ntc, 2 * ntc + 1)):
                nc.vector.bn_stats(
                    stats3[:, (mo * 8 + nt) * 6:(mo * 8 + nt) * 6 + 6],
                    pss[j][:])
                nc.scalar.activation(
                    f3[mo][:, nt * 512:(nt + 1) * 512].bitcast(f32r),
                    pss[j][:], Act.Identity, bias=b3_sb[:, mo:mo + 1])
    act3_cm.__exit__(None, None, None)

    # local (sum, sumsq) per channel -> AllReduce
    bnloc = per.tile([128, 8], f32, name="bnloc")
    mv3 = per.tile([128, 8], f32, name="mv3")
    for mo in range(4):
        nc.vector.bn_aggr(mv3[:, mo * 2:mo * 2 + 2],
                          stats3[:, mo * 48:(mo + 1) * 48])
        mu_ap = mv3[:, mo * 2:mo * 2 + 1]
        var_ap = mv3[:, mo * 2 + 1:mo * 2 + 2]
        # psum stats lack +b3: mu' = mu + b3 ; sumsq = N*(var + mu'^2)
        m2b = per.tile([128, 1], f32, name=f"m2b_{mo}")
        nc.vector.tensor_tensor(m2b[:], mu_ap, b3_sb[:, mo:mo + 1],
                                op=Alu.add)
        nc.scalar.activation(bnloc[:, mo * 2:mo * 2 + 1], m2b[:], Act.Identity,
                             scale=float(NPOS))
        nc.vector.scalar_tensor_tensor(
            bnloc[:, mo * 2 + 1:mo * 2 + 2], m2b[:], 1.0, m2b[:],
            op0=Alu.mult, op1=Alu.mult)
        nc.vector.tensor_tensor(bnloc[:, mo * 2 + 1:mo * 2 + 2],
                                bnloc[:, mo * 2 + 1:mo * 2 + 2], var_ap,
                                op=Alu.add)
        nc.scalar.activation(bnloc[:, mo * 2 + 1:mo * 2 + 2],
                             bnloc[:, mo * 2 + 1:mo * 2 + 2], Act.Identity,
                             scale=float(NPOS))
    cc3_in = dram.tile([128, 8], f32, name="cc3_in")
    cc3_out = dram.tile([128, 8], f32, name="cc3_out")
    nc.sync.dma_start(cc3_in[:], bnloc[:])
    nc.gpsimd.collective_compute("AllReduce", Alu.add, replica_groups=RG,
                                 ins=[cc3_in.opt()], outs=[cc3_out.opt()])
    g3 = per.tile([128, 8], f32, name="g3")
    nc.sync.dma_start(g3[:], cc3_out[:])
    sc3l, sh3l = [], []
    for mo in range(4):
        gmu = per.tile([128, 1], f32, name=f"gmu3_{mo}")
        nc.scalar.activation(gmu[:], g3[:, mo * 2:mo * 2 + 1], Act.Identity,
                             scale=1.0 / NTOT_POS)
        ge2 = per.tile([128, 1], f32, name=f"ge2_{mo}")
        nc.scalar.activation(ge2[:], g3[:, mo * 2 + 1:mo * 2 + 2], Act.Identity,
                             scale=1.0 / NTOT_POS)
        gvar = per.tile([128, 1], f32, name=f"gvar3_{mo}")
        nc.vector.tensor_tensor(gvar[:], gmu[:], gmu[:], op=Alu.mult)
        nc.vector.tensor_tensor(gvar[:], ge2[:], gvar[:], op=Alu.subtract)
        sc3, sh3 = bn_scale_shift(gvar[:], gmu[:], gamma3_sb[:, mo:mo + 1],
                                  beta3_sb[:, mo:mo + 1], f"bn3_{mo}")
        sc3l.append(sc3); sh3l.append(sh3)
    for nt in range(8):
        for mo in range(4):
            sl = f3[mo][:, nt * 512:(nt + 1) * 512]
            nc.scalar.activation(sl.bitcast(f32r), sl, Act.Relu,
                                 bias=sh3l[mo][:], scale=sc3l[mo][:])
    if debug:
        nc.sync.dma_start(io["dbg_f3h0"].ap(), f3[0][:])

    # ================ conv4 channel-major (for fg2) ================
    fg2 = per.tile([128, C], f32, name="fg2")  # [128, (mo:3, g:128)]
    with tc.tile_pool(name="f4pool", bufs=2) as f4pool:
        for mo in range(3):
            f4t = f4pool.tile([128, NPOS], f32, name="f4t", tag="f4t")
            for ntc in range(4):
                pss = [ps_mm.tile([128, 512], f32, name="mmps", tag="mmps")
                       for _ in range(2)]
                for kc in range(4):
                    for j, nt in enumerate((2 * ntc, 2 * ntc + 1)):
                        mm(pss[j][:], wblk(W4T, 3, kc, mo),
                           f3[kc][:, nt * 512:(nt + 1) * 512], kc == 0,
                           kc == 3)
                for j, nt in enumerate((2 * ntc, 2 * ntc + 1)):
                    nc.scalar.activation(f4t[:, nt * 512:(nt + 1) * 512],
                                         pss[j][:], Act.Identity,
                                         bias=b4_sb[:, mo:mo + 1])
            nc.vector.tensor_reduce(fg2[:, mo * 128:(mo + 1) * 128],
                                    f4t.rearrange("p (g i) -> p g i", i=32),
                                    axis=AX.X, op=Alu.max)
    ps_mm_cm.__exit__(None, None, None)
    if debug:
        nc.sync.dma_start(io["dbg_fg2"].ap(), fg2[:])

    # ================ conv4 position-major + H phase ================
    acc1 = per.tile([128, 32], f32, name="acc1")
    acc2 = per.tile([128, 32], f32, name="acc2")
    scrH = per.tile([128, C], f32, name="scrH")
    scrG = per.tile([128, C], f32, name="scrG")
    U_sb = per.tile([128, C], f32, name="U_sb")
    t1col = per.tile([128, 1], f32, name="t1col")
    uphase_cm = tc.tile_pool(name="uphase", bufs=1)
    uphase = uphase_cm.__enter__()
    # onesU[32*gp+n, t*128 + m] = 1 iff m == 4t+gp
    onesblk = uphase.tile([128, 4], f32, name="onesblk")
    nc.gpsimd.memset(onesblk[:], 1.0)
    nc.gpsimd.affine_select(onesblk[:], onesblk[:], pattern=[[-32, 4]],
                            compare_op=Alu.is_ge, fill=0.0, base=0,
                            channel_multiplier=1)
    nc.gpsimd.affine_select(onesblk[:], onesblk[:], pattern=[[32, 4]],
                            compare_op=Alu.is_ge, fill=0.0, base=31,
                            channel_multiplier=-1)
    onesU = uphase.tile([128, 32 * 128], f32, name="onesU")
    nc.vector.memset(onesU[:], 0.0)
    nc.vector.tensor_copy(onesU.bitcast(f32r), onesU[:])
    for t in range(32):
        nc.vector.tensor_copy(
            onesU[:, t * 128 + 4 * t:t * 128 + 4 * t + 4].bitcast(f32r),
            onesblk[:])
    alphar_ps = ps_c.tile([128, C], f32, name="alphar_ps", tag="cps")
    alpha_row = uphase.tile([1, C], f32, name="alpha_row")
    nc.sync.dma_start(alpha_row[:], io["alpha"].ap().unsqueeze(0))
    mm(alphar_ps[:], ones1x128[:], alpha_row[:], True, True, rep=False)
    alphar = uphase.tile([128, C], f32, name="alphar")
    nc.scalar.activation(alphar[:], alphar_ps[:], Act.Identity)

    with tc.tile_pool(name="psU", bufs=1, space="PSUM") as psU:
        Ups = psU.tile([128, C], f32, name="Ups", tag="hold")
        with tc.tile_pool(name="featpool", bufs=3) as featpool, \
             tc.tile_pool(name="psF", bufs=2, space="PSUM") as psF:
            for t in range(32):
                fps = psF.tile([128, C], f32, name="fps", tag="fps")
                for kc in range(4):
                    mm(fps[:], f3[kc][:, t * 128:(t + 1) * 128], w4rhs(kc),
                       kc == 0, False)
                mm(fps[:], ones1x128[:], b4row[:], False, True)  # + b4
                feat = featpool.tile([128, C], f32, name="feat", tag="feat")
                nc.scalar.activation(feat.bitcast(f32r), fps[:], Act.Identity)
                if debug and t == 0:
                    nc.sync.dma_start(io["dbg_feat0"].ap(), feat[:])
                hps = psF.tile([128, C], f32, name="hps", tag="hps", bufs=1)
                mm(hps[:], W_B[:, t * 128:(t + 1) * 128], feat[:],
                   True, True)
                nc.vector.scalar_tensor_tensor(
                    scrG[:], feat[:], Kc[:, t:t + 1], feat[:],
                    op0=Alu.mult, op1=Alu.mult, accum_out=acc1[:, t:t + 1])
                nc.vector.scalar_tensor_tensor(
                    scrH[:], feat[:], 1.0, hps[:],
                    op0=Alu.mult, op1=Alu.mult, accum_out=acc2[:, t:t + 1])
                hmkf = featpool.tile([128, C], f32, name="hmkf", tag="hmkf")
                nc.vector.scalar_tensor_tensor(
                    hmkf.bitcast(f32r), feat[:], -float(K), hps[:],
                    op0=Alu.mult, op1=Alu.add)
                mm(Ups[:], onesU[:, t * 128:(t + 1) * 128], hmkf[:],
                   t == 0, t == 31)
        nc.scalar.activation(U_sb[:], Ups[:], Act.Identity, accum_out=t1col[:])

    # V = alpha * U / (n*K)  (group-major), then transpose to channel-major
    V_sb = uphase.tile([128, C], f32, name="V_sb")
    nc.vector.scalar_tensor_tensor(V_sb[:], U_sb[:], 1.0 / (NPTS * K),
                                   alphar[:], op0=Alu.mult, op1=Alu.mult)
    Vc = per.tile([128, C], f32, name="Vc")
    for mo in range(3):
        pstv = ps_w.tile([128, 128], f32, name="wtps", tag="wtps")
        nc.tensor.transpose(pstv[:], V_sb[:, mo * 128:(mo + 1) * 128],
                            ident[:])
        nc.vector.tensor_copy(Vc[:, mo * 128:(mo + 1) * 128], pstv[:])
    uphase_cm.__exit__(None, None, None)
    a1r = per.tile([128, 1], f32, name="a1r")
    nc.vector.tensor_reduce(a1r[:], acc1[:], axis=AX.X, op=Alu.add)
    a2r = per.tile([128, 1], f32, name="a2r")
    nc.vector.tensor_reduce(a2r[:], acc2[:], axis=AX.X, op=Alu.add)
    t2col = per.tile([128, 1], f32, name="t2col")
    nc.vector.scalar_tensor_tensor(t2col[:], a2r[:], -2.0, a1r[:],
                                   op0=Alu.mult, op1=Alu.add)
    if debug:
        nc.sync.dma_start(io["dbg_U"].ap(), U_sb[:])

    # ================ P/Q matmuls + moments ================
    P_sb = per.tile([128, C], f32, name="P_sb")
    Q_sb = per.tile([128, C], f32, name="Q_sb")
    mom = per.tile([128, 17], f32, name="mom")
    scrP = per.tile([128, 128], f32, name="scrP")
    with tc.tile_pool(name="psQ", bufs=1, space="PSUM") as psQ:
        Pps = psQ.tile([128, C], f32, name="Pps", tag="holdP")
        Qps = psQ.tile([128, C], f32, name="Qps", tag="holdQ")
        for mo in range(3):
            for kc in range(3):
                mm(Pps[:, mo * 128:(mo + 1) * 128], wblk(WfT, 3, kc, mo),
                   fg2[:, kc * 128:(kc + 1) * 128], kc == 0, kc == 2,
                   rep=False)
                mm(Qps[:, mo * 128:(mo + 1) * 128], wblk(WfT, 3, 3 + kc, mo),
                   Vc[:, kc * 128:(kc + 1) * 128], kc == 0, kc == 2,
                   rep=False)
        for mo in range(3):
            nc.scalar.activation(P_sb[:, mo * 128:(mo + 1) * 128],
                                 Pps[:, mo * 128:(mo + 1) * 128],
                                 Act.Identity, bias=wbias[:, mo:mo + 1],
                                 accum_out=mom[:, mo:mo + 1])
            nc.scalar.activation(Q_sb[:, mo * 128:(mo + 1) * 128],
                                 Qps[:, mo * 128:(mo + 1) * 128], Act.Identity,
                                 accum_out=mom[:, 3 + mo:4 + mo])
            nc.scalar.activation(scrP[:], P_sb[:, mo * 128:(mo + 1) * 128],
                                 Act.Square, accum_out=mom[:, 6 + mo:7 + mo])
            nc.scalar.activation(scrP[:], Q_sb[:, mo * 128:(mo + 1) * 128],
                                 Act.Square, accum_out=mom[:, 9 + mo:10 + mo])
            nc.vector.scalar_tensor_tensor(
                scrP[:], P_sb[:, mo * 128:(mo + 1) * 128], 1.0,
                Q_sb[:, mo * 128:(mo + 1) * 128], op0=Alu.mult, op1=Alu.mult,
                accum_out=mom[:, 12 + mo:13 + mo])
    nc.vector.tensor_copy(mom[:, 15:16], t1col[:])
    nc.vector.tensor_copy(mom[:, 16:17], t2col[:])
    if debug:
        nc.sync.dma_start(io["dbg_P"].ap(), P_sb[:])
        nc.sync.dma_start(io["dbg_Q"].ap(), Q_sb[:])
        nc.sync.dma_start(io["dbg_mom"].ap(), mom[:])

    ccf_in = dram.tile([128, 17], f32, name="ccf_in")
    ccf_out = dram.tile([128, 17], f32, name="ccf_out")
    nc.sync.dma_start(ccf_in[:], mom[:])
    nc.gpsimd.collective_compute("AllReduce", Alu.add, replica_groups=RG,
                                 ins=[ccf_in.opt()], outs=[ccf_out.opt()])
    gmom = per.tile([128, 17], f32, name="gmom")
    nc.sync.dma_start(gmom[:], ccf_out[:])

    # T1/T2: partition-sum then broadcast back via K=1 matmul
    t12flat = per.tile([1, 256], f32, name="t12flat")
    nc.sync.dma_start(t12flat[:1].rearrange("1 (p c) -> 1 p c", c=2),
                      gmom[:, 15:17])
    t12 = per.tile([1, 2], f32, name="t12")
    nc.vector.tensor_reduce(t12[:],
                            t12flat.rearrange("1 (p c) -> 1 c p", c=2),
                            axis=AX.X, op=Alu.add)
    t12b_ps = ps_c.tile([128, 2], f32, name="t12b_ps", tag="cps")
    mm(t12b_ps[:], ones1x128[:], t12[:], True, True, rep=False)
    T1 = per.tile([128, 1], f32, name="T1")
    nc.vector.tensor_copy(T1[:], t12b_ps[:, 0:1])
    T2 = per.tile([128, 1], f32, name="T2")
    nc.vector.tensor_copy(T2[:], t12b_ps[:, 1:2])

    # s = 1 / (std + EPS); var = (T2 - T1^2/N) / (N-1)
    tA = per.tile([128, 1], f32, name="tA")
    nc.vector.tensor_tensor(tA[:], T1[:], T1[:], op=Alu.mult)
    tB = per.tile([128, 1], f32, name="tB")
    nc.vector.scalar_tensor_tensor(tB[:], tA[:], -1.0 / NTOT_DX, T2[:],
                                   op0=Alu.mult, op1=Alu.add)
    stdx = per.tile([128, 1], f32, name="stdx")
    nc.scalar.activation(stdx[:], tB[:], Act.Sqrt,
                         scale=1.0 / (NTOT_DX - 1))
    nc.vector.tensor_scalar(stdx[:], stdx[:], EPS, None, op0=Alu.add)
    s_col = per.tile([128, 1], f32, name="s_col")
    nc.vector.reciprocal(s_col[:], stdx[:])
    s2_col = per.tile([128, 1], f32, name="s2_col")
    nc.vector.tensor_tensor(s2_col[:], s_col[:], s_col[:], op=Alu.mult)

    # ================ BNf + output ================
    outsb = per.tile([128, C], f32, name="outsb")
    for mo in range(3):
        muf = per.tile([128, 1], f32, name=f"muf_{mo}")
        nc.vector.scalar_tensor_tensor(muf[:], gmom[:, 3 + mo:4 + mo],
                                       s_col[:], gmom[:, mo:mo + 1],
                                       op0=Alu.mult, op1=Alu.add)
        nc.scalar.activation(muf[:], muf[:], Act.Identity, scale=1.0 / NB)
        e2f = per.tile([128, 1], f32, name=f"e2f_{mo}")
        nc.vector.scalar_tensor_tensor(e2f[:], gmom[:, 12 + mo:13 + mo],
                                       s_col[:], gmom[:, 6 + mo:7 + mo],
                                       op0=Alu.mult, op1=Alu.add)
        nc.vector.scalar_tensor_tensor(e2f[:], gmom[:, 12 + mo:13 + mo],
                                       s_col[:], e2f[:],
                                       op0=Alu.mult, op1=Alu.add)
        nc.vector.scalar_tensor_tensor(e2f[:], gmom[:, 9 + mo:10 + mo],
                                       s2_col[:], e2f[:],
                                       op0=Alu.mult, op1=Alu.add)
        nc.scalar.activation(e2f[:], e2f[:], Act.Identity, scale=1.0 / NB)
        varf = per.tile([128, 1], f32, name=f"varf_{mo}")
        nc.vector.tensor_tensor(varf[:], muf[:], muf[:], op=Alu.mult)
        nc.vector.tensor_tensor(varf[:], e2f[:], varf[:], op=Alu.subtract)
        scf, shf = bn_scale_shift(varf[:], muf[:], gammaf_sb[:, mo:mo + 1],
                                  betaf_sb[:, mo:mo + 1], f"bnf_{mo}")
        zf = per.tile([128, 128], f32, name=f"zf_{mo}")
        nc.vector.scalar_tensor_tensor(zf[:],
                                       Q_sb[:, mo * 128:(mo + 1) * 128],
                                       s_col[:],
                                       P_sb[:, mo * 128:(mo + 1) * 128],
                                       op0=Alu.mult, op1=Alu.add)
        fused = per.tile([128, 128], f32, name=f"fused_{mo}")
        nc.scalar.activation(fused[:], zf[:], Act.Relu, bias=shf[:],
                             scale=scf[:])
        pst = ps_w.tile([128, 128], f32, name="wtps", tag="wtps")
        nc.tensor.transpose(pst[:], fused[:], ident[:])
        nc.vector.tensor_copy(outsb[:, mo * 128:(mo + 1) * 128], pst[:])

    nc.sync.dma_start(io["out"].ap(), outsb[:])
    ctx.close()


def _get_built():
    global _BUILT
    if _BUILT is None:
        _BUILT = _build(DEBUG)
    return _BUILT


def make_in_maps(inputs):
    pgf = np.ascontiguousarray(
        np.asarray(inputs["point_groups"], dtype=np.float32).reshape(
            1024, NPTS, 3))
    names = ["W1", "b1", "gamma1", "beta1", "W2", "b2", "W3", "b3",
             "gamma3", "beta3", "W4", "b4", "Wf", "bf", "gammaf", "betaf"]
    base = {n: np.ascontiguousarray(np.asarray(inputs[n], dtype=np.float32))
            for n in names}
    base["alpha"] = np.ascontiguousarray(
        np.asarray(inputs["alpha"], dtype=np.float32).reshape(C))
    base["beta_aff"] = np.ascontiguousarray(
        np.asarray(inputs["beta_aff"], dtype=np.float32).reshape(C))
    base["pg_full"] = pgf
    in_maps = []
    for c in range(NCORES):
        m = dict(base)
        m["pg"] = np.ascontiguousarray(pgf[c * G:(c + 1) * G])
        in_maps.append(m)
    return in_maps


def kernel(**inputs):
    from concourse.bass_utils import run_bass_kernel_spmd

    nc = _get_built()
    in_maps = make_in_maps(inputs)
    res = run_bass_kernel_spmd(nc, in_maps, list(range(NCORES)))
    full = np.concatenate([res.results[c]["out"] for c in range(NCORES)],
                          axis=0)
    return full.reshape(4, 256, C)



# revision 11
# speedup vs baseline: 1.3614x; 1.3614x over previous
"""Trainium2 Bass kernel for nn_EnhancedEncoder (gnn_message_passing).

Data-parallel over the 1024 flattened groups: 128 groups per core on 8 cores.
All intermediates stay in SBUF; big matmuls/activations in bf16, selection
and statistics math exact in fp32.  The KNN gather is reformulated as
block-diagonal adjacency matmuls; K_Norm reduces to count-weighted sums.

Structural notes vs the earlier version:
 - input-staging DMAs are issued before the comm warm-up collective so the
   gpsimd queue never blocks real work; post-collective DMAs ride sync.
 - all conv biases fold into BN shifts / the fusion-layer wbias, so every
   PSUM->SBUF copy is a pure cast and the K=1 bias matmuls disappear
   (b4 cancels in K_Norm entirely: all its moments are bias-invariant).
 - conv4 is computed once (position-major); fg2 = max over points comes from
   per-tile PE transposes + strided DVE max reductions directly off PSUM.
 - U (K_Norm mean path) is one matmul per tile with (cnt-K) weights.
 - the H loop is software-pipelined so PE never waits on the feat copy.

Position indexing per core: pos = t*128 + gp*32 + i  (t in [0,32), gp in
[0,4), i in [0,32)); group id g = 4*t + gp.
"""
import sys
from contextlib import ExitStack

for _p in ("/opt/trn_rl_repo",):
    if _p not in sys.path:
        sys.path.insert(0, _p)

import numpy as np

NCORES = 8
G = 128            # groups per core
NPTS = 32          # points per group
NPOS = G * NPTS    # 4096 positions per core
C = 384            # encoder channels
K = 8              # knn group size
EPS = 1e-5
NTOT_POS = 1024 * NPTS          # global positions (BN1/BN3 denominator)
NTOT_DX = 1024 * NPTS * K * C   # global dx element count (std denominator)
NB = 1024                       # global batch of groups (BNf denominator)
BIG_NEG = -1e30

_BUILT = None


def _build():
    import concourse.bacc as bacc
    import concourse.tile as tile
    from concourse import mybir

    f32 = mybir.dt.float32
    nc = bacc.Bacc("TRN2", target_bir_lowering=False, debug=False,
                   num_devices=NCORES)

    io = {}

    def din(name, shape):
        io[name] = nc.dram_tensor(name, shape, f32, kind="ExternalInput")

    din("pg", [G, NPTS, 3])
    din("pg_full", [1024, NPTS, 3])
    din("W1", [128, 3]); din("b1", [128])
    din("gamma1", [128]); din("beta1", [128])
    din("W2", [256, 128]); din("b2", [256])
    din("W3", [512, 512]); din("b3", [512])
    din("gamma3", [512]); din("beta3", [512])
    din("W4", [C, 512]); din("b4", [C])
    din("alpha", [C]); din("beta_aff", [C])
    din("Wf", [C, 2 * C]); din("bf", [C])
    din("gammaf", [C]); din("betaf", [C])
    io["out"] = nc.dram_tensor("out", [G, C], f32, kind="ExternalOutput")

    with tile.TileContext(nc) as tc:
        _emit(nc, tc, tile, mybir, io)
    nc.compile()
    return nc


def _emit(nc, tc, tile, mybir, io):
    f32 = mybir.dt.float32
    f32r = mybir.dt.float32r
    bf16 = mybir.dt.bfloat16
    Alu = mybir.AluOpType
    Act = mybir.ActivationFunctionType
    AX = mybir.AxisListType
    RG = [list(range(NCORES))]

    def mm(outap, lhsT, rhs, start, stop, rep=False):
        if rep:
            lhsT = lhsT.bitcast(f32r)
            rhs = rhs.bitcast(f32r)
        nc.tensor.matmul(outap, lhsT, rhs, start=start, stop=stop,
                         skip_group_check=True)

    ctx = ExitStack()
    per = ctx.enter_context(tc.tile_pool(name="per", bufs=1))
    dram = ctx.enter_context(tc.tile_pool(name="dram", bufs=1, space="DRAM"))

    # ---------------- constants ----------------
    ident = per.tile([128, 128], f32, name="ident")
    nc.gpsimd.memset(ident[:], 1.0)
    nc.gpsimd.affine_select(ident[:], ident[:], pattern=[[1, 128]],
                            compare_op=Alu.is_equal, fill=0.0, base=0,
                            channel_multiplier=-1)
    identb = per.tile([128, 128], bf16, name="identb")
    nc.scalar.copy(identb[:], ident[:])
    ones1x128 = per.tile([1, 128], f32, name="ones1x128")
    nc.gpsimd.memset(ones1x128[:], 1.0)
    ones128x1 = per.tile([128, 1], f32, name="ones128x1")
    nc.gpsimd.memset(ones128x1[:], 1.0)
    eps_col = per.tile([128, 1], f32, name="eps_col")
    nc.gpsimd.memset(eps_col[:], EPS)
    # onesblk[32gp+n, col] = 1 iff col == gp   (for the Wp diagonal blocks)
    onesblk = per.tile([128, 4], f32, name="onesblk")
    nc.gpsimd.memset(onesblk[:], 1.0)
    nc.gpsimd.affine_select(onesblk[:], onesblk[:], pattern=[[-32, 4]],
                            compare_op=Alu.is_ge, fill=0.0, base=0,
                            channel_multiplier=1)
    nc.gpsimd.affine_select(onesblk[:], onesblk[:], pattern=[[32, 4]],
                            compare_op=Alu.is_ge, fill=0.0, base=31,
                            channel_multiplier=-1)

    # ---------------- input staging DMAs (before any collective) ----------
    sel_a = tc.tile_pool(name="sel_a", bufs=1)
    sa = sel_a.__enter__()
    # BN1 global input moments source: full pg on every core
    pgm = sa.tile([128, 768], f32, name="pgm")   # [128, (jj:8, i:32, c:3)]
    nc.gpsimd.dma_start(pgm[:], io["pg_full"].ap().rearrange(
        "(p jj) i c -> p (jj i c)", p=128).opt())
    # pgA[32gp+i, (t, c)] = pg[4t+gp, i, c]
    pgA = sa.tile([128, 96], f32, name="pgA")
    pgr4 = io["pg"].ap().rearrange("(t gp) i c -> gp i t c", gp=4)
    for gp in range(4):
        nc.gpsimd.dma_start(
            pgA[32 * gp:32 * gp + 32].rearrange("i (t c) -> i t c", t=32).opt(),
            pgr4[gp].opt())
    # pgstage[gp, (t, m, c)] in DRAM, contiguous per gp
    pgstage = dram.tile([4, NPTS * 32 * 3], f32, name="pgstage")
    pgr5 = io["pg"].ap().rearrange("(t gp) m c -> gp t m c", gp=4)
    for gp in range(4):
        nc.gpsimd.dma_start(
            pgstage[gp:gp + 1].rearrange("1 (t m c) -> 1 t m c", t=32,
                                         m=32).opt(),
            pgr5[gp].opt())
    # pgB[32gp+n, (t, m, c)] = pg[4t+gp, m, c]   (replicated over n)
    pgB = sa.tile([128, 3072], f32, name="pgB")
    for gp in range(4):
        nc.gpsimd.dma_start(
            pgB[32 * gp:32 * gp + 32],
            pgstage[gp:gp + 1].broadcast_to([32, 3072]))

    # ---------------- dummy collective (comm warm-up) ----------------
    warm_in = dram.tile([128, 1], f32, name="warm_in")
    warm_out = dram.tile([128, 1], f32, name="warm_out")
    nc.sync.dma_start(warm_in[:], ones128x1[:])
    nc.gpsimd.collective_compute("AllReduce", Alu.add, replica_groups=RG,
                                 ins=[warm_in.opt()], outs=[warm_out.opt()])

    # setup-phase PSUM pools (closed before the conv pipeline opens)
    ps_w_cm = tc.tile_pool(name="ps_w", bufs=2, space="PSUM")
    ps_w = ps_w_cm.__enter__()
    ps_c_cm = tc.tile_pool(name="ps_c", bufs=2, space="PSUM")
    ps_c = ps_c_cm.__enter__()

    # ---------------- load weights + vectors (all DMAs on sync) ----------
    vstage_cm = tc.tile_pool(name="vstage", bufs=2)
    vstage = vstage_cm.__enter__()

    def vload(name, n):
        nt = n // 128
        st = vstage.tile([4, 128], f32, name=name + "_st", tag="vst")
        nc.sync.dma_start(st[:nt, :],
                          io[name].ap().rearrange("(a p) -> a p", p=128))
        pst = ps_c.tile([128, 4], f32, name=name + "_ps", tag="cps")
        nc.tensor.transpose(pst[:, :nt], st[:nt, :], ident[:nt, :nt])
        t = per.tile([128, nt], f32, name=name + "_sb")
        nc.vector.tensor_copy(t[:], pst[:, :nt])
        return t

    b1_sb = vload("b1", 128)
    gamma1_sb = vload("gamma1", 128)
    beta1_sb = vload("beta1", 128)
    b2_sb = vload("b2", 256)
    b3_sb = vload("b3", 512)
    gamma3_sb = vload("gamma3", 512)
    beta3_sb = vload("beta3", 512)
    b4_sb = vload("b4", C)
    bf_sb = vload("bf", C)
    gammaf_sb = vload("gammaf", C)
    betaf_sb = vload("betaf", C)
    betaaff_sb = vload("beta_aff", C)
    alpha_sb = vload("alpha", C)
    W1_sb = per.tile([128, 3], f32, name="W1_sb")
    nc.sync.dma_start(W1_sb[:], io["W1"].ap())
    vstage_cm.__exit__(None, None, None)
    W1T = per.tile([3, 128], f32, name="W1T")
    nc.sync.dma_start(W1T[:].bitcast(f32r),
                      io["W1"].ap().rearrange("o c -> c o").bitcast(f32r))

    # big weights: DMA raw, transpose on PE, store bf16 blocks
    wts = {}
    with tc.tile_pool(name="wraw", bufs=1) as wraw:
        for name, rows, cols in [("W2", 256, 128), ("W3", 512, 512),
                                 ("W4", C, 512), ("Wf", C, 2 * C)]:
            nr, kg = rows // 128, cols // 128
            raw = wraw.tile([128, nr * cols], f32, name=name + "_raw",
                            tag="wraw")
            nc.sync.dma_start(
                raw.rearrange("p (rt c) -> p rt c", rt=nr),
                io[name].ap().rearrange("(rt p) c -> p rt c", p=128))
            wt = per.tile([128, kg * nr * 128], bf16, name=name + "T")
            for kc in range(kg):
                for mo in range(nr):
                    pst = ps_w.tile([128, 128], f32, name="wtps", tag="wtps")
                    nc.tensor.transpose(
                        pst[:],
                        raw[:, mo * cols + kc * 128:mo * cols + kc * 128 + 128],
                        ident[:])
                    dst = wt[:, (kc * nr + mo) * 128:(kc * nr + mo) * 128 + 128]
                    if (kc + mo) % 2 == 0:
                        nc.vector.tensor_copy(dst, pst[:])
                    else:
                        nc.scalar.copy(dst, pst[:])
            wts[name] = wt
    W2T, W3T, W4T, WfT = wts["W2"], wts["W3"], wts["W4"], wts["Wf"]

    def wblk(wt, nr, kc, mo):
        return wt[:, (kc * nr + mo) * 128:(kc * nr + mo) * 128 + 128]

    def w4rhs(kc):  # pos-major rhs [128, 384] = blocks (kc, mo=0..2)
        return W4T[:, kc * 3 * 128:(kc * 3 + 3) * 128]

    # bf16 copies of the small bias columns used in the fold matmuls
    b2c16 = per.tile([128, 2], bf16, name="b2c16")
    nc.scalar.copy(b2c16[:], b2_sb[:])
    b4c16 = per.tile([128, 3], bf16, name="b4c16")
    nc.scalar.copy(b4c16[:], b4_sb[:])
    bac16 = per.tile([128, 3], bf16, name="bac16")
    nc.scalar.copy(bac16[:], betaaff_sb[:])

    # wbias = Wf[:, :C] @ b4 + Wf[:, C:] @ beta_aff + bf   (channel-major)
    wbias_ps = ps_c.tile([128, 3], f32, name="wbias_ps", tag="cps")
    for mo in range(3):
        for kc in range(3):
            mm(wbias_ps[:, mo:mo + 1], wblk(WfT, 3, kc, mo),
               b4c16[:, kc:kc + 1], kc == 0, False)
            mm(wbias_ps[:, mo:mo + 1], wblk(WfT, 3, 3 + kc, mo),
               bac16[:, kc:kc + 1], False, kc == 2)
    wbias = per.tile([128, 3], f32, name="wbias")
    nc.vector.tensor_tensor(wbias[:], wbias_ps[:], bf_sb[:], op=Alu.add)

    # b3p = b3 + sum_kc W3blk[kc].T @ b2[kc%2]  (f2/fg stored without b2)
    b3p_ps = ps_c.tile([128, 4], f32, name="b3p_ps", tag="cps")
    for mo in range(4):
        for kc in range(4):
            mm(b3p_ps[:, mo:mo + 1], wblk(W3T, 4, kc, mo),
               b2c16[:, (kc % 2):(kc % 2) + 1], kc == 0, kc == 3)
    b3p = per.tile([128, 4], f32, name="b3p")
    nc.vector.tensor_tensor(b3p[:], b3p_ps[:], b3_sb[:], op=Alu.add)

    def bn_scale_shift(var_ap, mu_ap, gam_ap, bet_ap, pref, n=1):
        std = per.tile([128, n], f32, name=pref + "_std")
        nc.scalar.activation(std[:], var_ap, Act.Sqrt, bias=eps_col[:])
        rstd = per.tile([128, n], f32, name=pref + "_rstd")
        nc.vector.reciprocal(rstd[:], std[:])
        sc = per.tile([128, n], f32, name=pref + "_sc")
        nc.vector.tensor_tensor(sc[:], rstd[:], gam_ap, op=Alu.mult)
        sh = per.tile([128, n], f32, name=pref + "_sh")
        nc.vector.tensor_tensor(sh[:], mu_ap, sc[:], op=Alu.mult)
        nc.vector.tensor_tensor(sh[:], bet_ap, sh[:], op=Alu.subtract)
        return sc, sh

    # ---------------- BN1 scale/shift from global input moments ----------
    mcols = sa.tile([128, 12], f32, name="mcols")
    pv = pgm.rearrange("p (j c) -> p j c", c=3)
    scr256 = sa.tile([128, 256], f32, name="scr256")
    for i in range(3):
        for j in range(3):
            nc.vector.scalar_tensor_tensor(
                scr256[:], pv[:, :, i], 1.0, pv[:, :, j],
                op0=Alu.mult, op1=Alu.mult,
                accum_out=mcols[:, 3 * i + j:3 * i + j + 1])
        nc.vector.tensor_reduce(mcols[:, 9 + i:10 + i], pv[:, :, i],
                                axis=AX.X, op=Alu.add)
    m12 = ps_c.tile([1, 12], f32, name="m12", tag="cps")
    mm(m12[:], ones128x1[:], mcols[:], True, True)
    m12s = sa.tile([1, 12], f32, name="m12s")
    nc.scalar.activation(m12s[:], m12[:], Act.Identity, scale=1.0 / NTOT_POS)
    M2sb = sa.tile([3, 3], f32, name="M2sb")
    nc.sync.dma_start(M2sb[:],
                      m12s[:1, :9].rearrange("1 (i j) -> 1 i j", i=3))
    mu3 = sa.tile([3, 1], f32, name="mu3")
    nc.sync.dma_start(mu3[:], m12s[:1, 9:12])

    m1ps = ps_c.tile([128, 1], f32, name="m1ps", tag="cps")
    mm(m1ps[:], W1T[:], mu3[:], True, True)   # W1 @ mu_p
    mvec = per.tile([128, 1], f32, name="mvec")
    nc.vector.tensor_copy(mvec[:], m1ps[:])
    wmps = ps_c.tile([128, 3], f32, name="wmps", tag="cps")
    mm(wmps[:], W1T[:], M2sb[:], True, True)  # W1 @ M2
    e2raw = per.tile([128, 1], f32, name="e2raw")
    scr3 = per.tile([128, 3], f32, name="scr3")
    nc.vector.scalar_tensor_tensor(scr3[:], wmps[:], 1.0, W1_sb[:],
                                   op0=Alu.mult, op1=Alu.mult,
                                   accum_out=e2raw[:])
    mu1 = per.tile([128, 1], f32, name="mu1")
    nc.vector.tensor_tensor(mu1[:], mvec[:], b1_sb[:], op=Alu.add)
    t_a = per.tile([128, 1], f32, name="t_a")
    nc.vector.tensor_tensor(t_a[:], mvec[:], b1_sb[:], op=Alu.mult)
    e2 = per.tile([128, 1], f32, name="e2")
    nc.vector.scalar_tensor_tensor(e2[:], t_a[:], 2.0, e2raw[:],
                                   op0=Alu.mult, op1=Alu.add)
    nc.vector.tensor_tensor(t_a[:], b1_sb[:], b1_sb[:], op=Alu.mult)
    nc.vector.tensor_tensor(e2[:], e2[:], t_a[:], op=Alu.add)
    nc.vector.tensor_tensor(t_a[:], mu1[:], mu1[:], op=Alu.mult)
    var1 = per.tile([128, 1], f32, name="var1")
    nc.vector.tensor_tensor(var1[:], e2[:], t_a[:], op=Alu.subtract)
    sc1, sh1 = bn_scale_shift(var1[:], mu1[:], gamma1_sb[:], beta1_sb[:],
                              "bn1")
    # fold conv bias into the shift: f1h = relu(sc1*(W1x) + (sc1*b1 + sh1))
    sh1b = per.tile([128, 1], f32, name="sh1b")
    nc.vector.tensor_tensor(sh1b[:], sc1[:], b1_sb[:], op=Alu.mult)
    nc.vector.tensor_tensor(sh1b[:], sh1b[:], sh1[:], op=Alu.add)

    # ---------------- x0 (position-major coordinates) --------------------
    # x0[c, t*128+32gp+i] = pg[4t+gp, i, c]  via PE transpose of pgA2
    pgA2 = sa.tile([128, 96], f32, name="pgA2")  # [32gp+i, (c, t)]
    nc.vector.tensor_copy(
        pgA2.rearrange("p (c t) -> p c t", c=3),
        pgA.rearrange("p (t c) -> p c t", c=3))
    tps0 = ps_w.tile([128, 128], f32, name="wtps", tag="wtps")
    nc.tensor.transpose(tps0[0:96, :], pgA2[:], ident[:])
    T_sb = sa.tile([96, 128], f32, name="T_sb")
    nc.vector.tensor_copy(T_sb[:], tps0[0:96, :])
    pgT_dram = dram.tile([3, NPOS], f32, name="pgT_dram")
    for cdim in range(3):
        nc.sync.dma_start(
            pgT_dram[cdim:cdim + 1].rearrange("1 (t q) -> 1 t q", t=32).opt(),
            T_sb[32 * cdim:32 * cdim + 32])

    # ---------------- distance keys (exact fp32) -------------------------
    # negkey[32gp+n, t*32+m] = sum_c (pgA[.,t,c] - 0.5*pgB_c)*pgB_c
    scr1 = sa.tile([128, 1024], f32, name="scr1")
    negkey = per.tile([128, 1024], f32, name="negkey")
    for cdim in range(3):
        pgB_c = pgB.rearrange("p (t m c) -> p t m c", t=32, m=32)[:, :, :, cdim]
        pgA_c = pgA.rearrange("p (t c) -> p t c", c=3)[:, :, cdim] \
            .unsqueeze(2).broadcast_to([128, 32, 32])
        dst = scr1[:] if cdim > 0 else negkey[:]
        dstv = dst.rearrange("p (t m) -> p t m", t=32)
        nc.vector.scalar_tensor_tensor(dstv, pgB_c, -0.5, pgA_c,
                                       op0=Alu.mult, op1=Alu.add)
        nc.vector.tensor_tensor(dstv, dstv, pgB_c, op=Alu.mult)
        if cdim > 0:
            nc.vector.tensor_tensor(negkey[:], negkey[:], scr1[:],
                                    op=Alu.add)

    # close setup PSUM pools + staging (LIFO) before the conv pipeline opens
    ps_c_cm.__exit__(None, None, None)
    ps_w_cm.__exit__(None, None, None)
    sel_a.__exit__(None, None, None)

    # ---------------- conv pipeline PSUM pool ----------------------------
    ps_mm_cm = tc.tile_pool(name="ps_mm", bufs=6, space="PSUM")
    ps_mm = ps_mm_cm.__enter__()
    act2_cm = tc.tile_pool(name="act2", bufs=1)
    act2 = act2_cm.__enter__()
    act1_cm = tc.tile_pool(name="act1", bufs=1)
    act1 = act1_cm.__enter__()

    # ---------------- conv1 (f32r, exact input) --------------------------
    x0 = act1.tile([3, NPOS], f32, name="x0")
    nc.sync.dma_start(x0[:].bitcast(f32r), pgT_dram[:].bitcast(f32r))
    f1h = act1.tile([128, NPOS], bf16, name="f1h")
    for nt in range(8):
        ps = ps_mm.tile([128, 512], f32, name="mmps", tag="mmps")
        mm(ps[:], W1T[:], x0[:, nt * 512:(nt + 1) * 512], True, True,
           rep=True)
        nc.scalar.activation(f1h[:, nt * 512:(nt + 1) * 512],
                             ps[:], Act.Relu, bias=sh1b[:], scale=sc1[:])

    # ---------------- conv2 + fg ----------------------------------------
    # f2 stored WITHOUT b2 (folded into b3p); fg = max over points of f2_nob
    f2 = [act2.tile([128, NPOS], bf16, name=f"f2_{mo}") for mo in range(2)]
    fg = per.tile([128, 256], bf16, name="fg")  # [128, (mo:2, g:128)]
    for mo in range(2):
        for nt in range(8):
            ps = ps_mm.tile([128, 512], f32, name="mmps", tag="mmps")
            mm(ps[:], wblk(W2T, 2, 0, mo), f1h[:, nt * 512:(nt + 1) * 512],
               True, True)
            nc.vector.tensor_reduce(
                fg[:, mo * 128 + nt * 16:mo * 128 + (nt + 1) * 16],
                ps.rearrange("p (g i) -> p g i", i=32), axis=AX.X, op=Alu.max)
            if nt % 2 == 0:
                nc.scalar.copy(f2[mo][:, nt * 512:(nt + 1) * 512], ps[:])
            else:
                nc.vector.tensor_copy(f2[mo][:, nt * 512:(nt + 1) * 512],
                                      ps[:])
    act1_cm.__exit__(None, None, None)

    # ---------------- top-8 selection + W_B / Wp -------------------------
    top8 = per.tile([128, 8], f32, name="top8")
    repl = per.tile([128, 1024], f32, name="repl")
    for t in range(32):
        nc.vector.max(top8[:], negkey[:, t * 32:(t + 1) * 32])
        nc.vector.match_replace(repl[:, t * 32:(t + 1) * 32], top8[:],
                                negkey[:, t * 32:(t + 1) * 32], BIG_NEG)
    A2 = per.tile([128, 1024], bf16, name="A2")
    nc.vector.tensor_scalar(A2[:], repl[:], BIG_NEG, None, op0=Alu.is_equal)

    A2T = per.tile([128, 1024], bf16, name="A2T")
    nc.vector.transpose(A2T[:], A2[:])
    # Kc[32gp+m, t] = K + sum_n A[n, m]
    Kc = per.tile([128, 32], f32, name="Kc")
    nc.vector.tensor_reduce(Kc[:],
                            A2T.rearrange("p (t n) -> p t n", t=32),
                            axis=AX.X, op=Alu.add)
    nc.vector.tensor_scalar(Kc[:], Kc[:], float(K), None, op0=Alu.add)
    sqKc = per.tile([128, 32], f32, name="sqKc")
    nc.scalar.activation(sqKc[:], Kc[:], Act.Sqrt)
    Kc2 = per.tile([128, 32], f32, name="Kc2")
    nc.vector.tensor_scalar(Kc2[:], Kc[:], float(2 * K), None,
                            op0=Alu.subtract)

    # W_B[32gp+m, t*128+32gp+n] = A2T[32gp+m, t*32+n]  (block-diag lhsT)
    W_B = per.tile([128, NPOS], bf16, name="W_B")
    nc.vector.memset(W_B[:], 0.0)
    for gp in range(4):
        nc.vector.tensor_copy(
            W_B[32 * gp:32 * gp + 32].rearrange(
                "p (t q) -> p t q", t=32)[:, :, 32 * gp:32 * gp + 32],
            A2T[32 * gp:32 * gp + 32].rearrange("p (t n) -> p t n", t=32))
    # Wp[32gp+n, t*128+m] = (cnt-K)[pos] iff m == 4t+gp
    Wp = per.tile([128, NPOS], bf16, name="Wp")
    nc.vector.memset(Wp[:], 0.0)
    for t in range(32):
        nc.vector.scalar_tensor_tensor(
            Wp[:, t * 128 + 4 * t:t * 128 + 4 * t + 4],
            onesblk[:], Kc2[:, t:t + 1], onesblk[:],
            op0=Alu.mult, op1=Alu.mult)

    # ---------------- conv3 + BN3 stats ----------------------------------
    f3 = [per.tile([128, NPOS], bf16, name=f"f3_{mo}") for mo in range(4)]
    stats3 = per.tile([128, 32 * 6], f32, name="stats3")
    for mo in range(4):
        for nt in range(8):
            ps = ps_mm.tile([128, 512], f32, name="mmps", tag="mmps")
            for kc in range(4):
                if kc < 2:
                    rhs = fg[:, kc * 128 + nt * 16:
                             kc * 128 + (nt + 1) * 16] \
                        .unsqueeze(2).broadcast_to([128, 16, 32])
                else:
                    rhs = f2[kc - 2][:, nt * 512:(nt + 1) * 512]
                mm(ps[:], wblk(W3T, 4, kc, mo), rhs, kc == 0, kc == 3)
            nc.vector.bn_stats(
                stats3[:, (mo * 8 + nt) * 6:(mo * 8 + nt) * 6 + 6], ps[:])
            if (mo + nt) % 2 == 0:
                nc.scalar.copy(f3[mo][:, nt * 512:(nt + 1) * 512], ps[:])
            else:
                nc.vector.tensor_copy(f3[mo][:, nt * 512:(nt + 1) * 512],
                                      ps[:])
    act2_cm.__exit__(None, None, None)

    # local (sum, sumsq) per channel -> AllReduce
    bnloc = per.tile([128, 8], f32, name="bnloc")
    mv3 = per.tile([128, 8], f32, name="mv3")
    for mo in range(4):
        nc.vector.bn_aggr(mv3[:, mo * 2:mo * 2 + 2],
                          stats3[:, mo * 48:(mo + 1) * 48])
        mu_ap = mv3[:, mo * 2:mo * 2 + 1]
        var_ap = mv3[:, mo * 2 + 1:mo * 2 + 2]
        # psum stats lack +b3p: mu' = mu + b3p ; sumsq = N*(var + mu'^2)
        m2b = per.tile([128, 1], f32, name=f"m2b_{mo}")
        nc.vector.tensor_tensor(m2b[:], mu_ap, b3p[:, mo:mo + 1],
                                op=Alu.add)
        nc.scalar.activation(bnloc[:, mo * 2:mo * 2 + 1], m2b[:], Act.Identity,
                             scale=float(NPOS))
        nc.vector.scalar_tensor_tensor(
            bnloc[:, mo * 2 + 1:mo * 2 + 2], m2b[:], 1.0, m2b[:],
            op0=Alu.mult, op1=Alu.mult)
        nc.vector.tensor_tensor(bnloc[:, mo * 2 + 1:mo * 2 + 2],
                                bnloc[:, mo * 2 + 1:mo * 2 + 2], var_ap,
                                op=Alu.add)
        nc.scalar.activation(bnloc[:, mo * 2 + 1:mo * 2 + 2],
                             bnloc[:, mo * 2 + 1:mo * 2 + 2], Act.Identity,
                             scale=float(NPOS))
    cc3_in = dram.tile([128, 8], f32, name="cc3_in")
    cc3_out = dram.tile([128, 8], f32, name="cc3_out")
    nc.sync.dma_start(cc3_in[:], bnloc[:])
    nc.gpsimd.collective_compute("AllReduce", Alu.add, replica_groups=RG,
                                 ins=[cc3_in.opt()], outs=[cc3_out.opt()])

    # -------- work that overlaps the BN3 collective --------
    alphar_ps = ps_mm.tile([128, C], f32, name="alphar_ps", tag="alph",
                           bufs=1)
    alpha_row = per.tile([1, C], f32, name="alpha_row")
    nc.sync.dma_start(alpha_row[:], io["alpha"].ap().unsqueeze(0))
    mm(alphar_ps[:], ones1x128[:], alpha_row[:], True, True)
    alphar = per.tile([128, C], f32, name="alphar")
    nc.scalar.copy(alphar[:], alphar_ps[:])

    g3 = per.tile([128, 8], f32, name="g3")
    nc.sync.dma_start(g3[:], cc3_out[:])
    sc3l, sh3l = [], []
    for mo in range(4):
        gmu = per.tile([128, 1], f32, name=f"gmu3_{mo}")
        nc.scalar.activation(gmu[:], g3[:, mo * 2:mo * 2 + 1], Act.Identity,
                             scale=1.0 / NTOT_POS)
        ge2 = per.tile([128, 1], f32, name=f"ge2_{mo}")
        nc.scalar.activation(ge2[:], g3[:, mo * 2 + 1:mo * 2 + 2], Act.Identity,
                             scale=1.0 / NTOT_POS)
        gvar = per.tile([128, 1], f32, name=f"gvar3_{mo}")
        nc.vector.tensor_tensor(gvar[:], gmu[:], gmu[:], op=Alu.mult)
        nc.vector.tensor_tensor(gvar[:], ge2[:], gvar[:], op=Alu.subtract)
        sc3, sh3 = bn_scale_shift(gvar[:], gmu[:], gamma3_sb[:, mo:mo + 1],
                                  beta3_sb[:, mo:mo + 1], f"bn3_{mo}")
        # fold b3p: shift' = sc3*b3p + sh3
        sh3b = per.tile([128, 1], f32, name=f"sh3b_{mo}")
        nc.vector.tensor_tensor(sh3b[:], sc3[:], b3p[:, mo:mo + 1],
                                op=Alu.mult)
        nc.vector.tensor_tensor(sh3b[:], sh3b[:], sh3[:], op=Alu.add)
        sc3l.append(sc3); sh3l.append(sh3b)

    # ReLU3 in place, chunk-major so the H loop can start early.
    # mo 0,1 on scalar; mo 2,3 on vector (2 ops each) for engine balance.
    for nt in range(4):
        for mo in range(4):
            sl = f3[mo][:, nt * 1024:(nt + 1) * 1024]
            if mo < 2:
                nc.scalar.activation(sl, sl, Act.Relu,
                                     bias=sh3l[mo][:], scale=sc3l[mo][:])
            else:
                nc.vector.scalar_tensor_tensor(
                    sl, sl, sc3l[mo][:],
                    sh3l[mo][:].broadcast_to([128, 1024]),
                    op0=Alu.mult, op1=Alu.add)
                nc.vector.tensor_scalar(sl, sl, 0.0, None, op0=Alu.max)
    ps_mm_cm.__exit__(None, None, None)

    # ---------------- conv4 position-major + H phase ---------------------
    accA = per.tile([128, 32], f32, name="accA")
    accB = per.tile([128, 32], f32, name="accB")
    fg2 = per.tile([128, C], bf16, name="fg2")  # [128ch, (mo:3, g:128)]
    U_sb = per.tile([128, C], f32, name="U_sb")
    t1col = per.tile([128, 1], f32, name="t1col")

    psU_cm = tc.tile_pool(name="psU", bufs=1, space="PSUM")
    psU = psU_cm.__enter__()
    Ups = psU.tile([128, C], f32, name="Ups", tag="hold")
    featpool_cm = tc.tile_pool(name="featpool", bufs=3)
    featpool = featpool_cm.__enter__()
    scrpool_cm = tc.tile_pool(name="scrpool", bufs=2)
    scrpool = scrpool_cm.__enter__()
    psF_cm = tc.tile_pool(name="psF", bufs=2, space="PSUM")
    psF = psF_cm.__enter__()
    psT_cm = tc.tile_pool(name="psT", bufs=2, space="PSUM")
    psT = psT_cm.__enter__()
    psH_cm = tc.tile_pool(name="psH", bufs=2, space="PSUM")
    psH = psH_cm.__enter__()

    def emit_fps(t):
        fps = psF.tile([128, C], f32, name="fps", tag="fps")
        for kc in range(4):
            mm(fps[:], f3[kc][:, t * 128:(t + 1) * 128], w4rhs(kc),
               kc == 0, kc == 3)
        return fps

    fps_t = emit_fps(0)
    for t in range(32):
        feat = featpool.tile([128, C], bf16, name="feat", tag="feat")
        nc.scalar.copy(feat[:], fps_t[:])
        if t < 31:
            fps_t = emit_fps(t + 1)
        # h = A^T-block @ feat  (for the t2 cross term)
        hps = psH.tile([128, C], f32, name="hps", tag="hps")
        mm(hps[:], W_B[:, t * 128:(t + 1) * 128], feat[:], True, True)
        # U += (cnt-K)-weighted feat
        mm(Ups[:], Wp[:, t * 128:(t + 1) * 128], feat[:], t == 0, t == 31)
        # transposes of feat -> channel-major, for fg2
        tp = psT.tile([128, C], bf16, name="tp", tag="tp")
        for mo in range(3):
            nc.tensor.transpose(tp[:, mo * 128:(mo + 1) * 128],
                                feat[:, mo * 128:(mo + 1) * 128], identb[:])
        # accumulations
        scrA = scrpool.tile([128, C], bf16, name="scrA", tag="scrA")
        nc.scalar.activation(scrA[:], feat[:], Act.Square,
                             scale=sqKc[:, t:t + 1],
                             accum_out=accA[:, t:t + 1])
        scrB = scrpool.tile([128, C], f32, name="scrB", tag="scrB")
        nc.vector.scalar_tensor_tensor(scrB[:], feat[:], 1.0, hps[:],
                                       op0=Alu.mult, op1=Alu.mult,
                                       accum_out=accB[:, t:t + 1])
        for mo in range(3):
            nc.vector.tensor_reduce(
                fg2[:, mo * 128 + 4 * t:mo * 128 + 4 * t + 4],
                tp[:, mo * 128:(mo + 1) * 128]
                .rearrange("p (gp i) -> p gp i", gp=4),
                axis=AX.X, op=Alu.max)
    nc.scalar.activation(U_sb[:], Ups[:], Act.Identity, accum_out=t1col[:])
    psH_cm.__exit__(None, None, None)
    psT_cm.__exit__(None, None, None)
    psF_cm.__exit__(None, None, None)
    scrpool_cm.__exit__(None, None, None)
    featpool_cm.__exit__(None, None, None)
    psU_cm.__exit__(None, None, None)

    # ---------------- tail: V/Vc, P/Q, moments, BNf ----------------------
    ps_t_cm = tc.tile_pool(name="ps_t", bufs=2, space="PSUM")
    ps_t = ps_t_cm.__enter__()

    V_sb = per.tile([128, C], f32, name="V_sb")
    nc.vector.scalar_tensor_tensor(V_sb[:], U_sb[:], 1.0 / (NPTS * K),
                                   alphar[:], op0=Alu.mult, op1=Alu.mult)
    Vc = per.tile([128, C], bf16, name="Vc")
    for mo in range(3):
        pstv = ps_t.tile([128, 128], f32, name="wtps", tag="wtps")
        nc.tensor.transpose(pstv[:], V_sb[:, mo * 128:(mo + 1) * 128],
                            ident[:])
        nc.vector.tensor_copy(Vc[:, mo * 128:(mo + 1) * 128], pstv[:])
    a1r = per.tile([128, 1], f32, name="a1r")
    nc.vector.tensor_reduce(a1r[:], accA[:], axis=AX.X, op=Alu.add)
    b1r = per.tile([128, 1], f32, name="b1r")
    nc.vector.tensor_reduce(b1r[:], accB[:], axis=AX.X, op=Alu.add)
    t2col = per.tile([128, 1], f32, name="t2col")
    nc.vector.scalar_tensor_tensor(t2col[:], b1r[:], -2.0, a1r[:],
                                   op0=Alu.mult, op1=Alu.add)

    # P/Q matmuls + moments
    P_sb = per.tile([128, C], f32, name="P_sb")
    Q_sb = per.tile([128, C], f32, name="Q_sb")
    mom = per.tile([128, 17], f32, name="mom")
    scrP = per.tile([128, 128], f32, name="scrP")
    with tc.tile_pool(name="psQ", bufs=1, space="PSUM") as psQ:
        Pps = psQ.tile([128, C], f32, name="Pps", tag="holdP")
        Qps = psQ.tile([128, C], f32, name="Qps", tag="holdQ")
        for mo in range(3):
            for kc in range(3):
                mm(Pps[:, mo * 128:(mo + 1) * 128], wblk(WfT, 3, kc, mo),
                   fg2[:, kc * 128:(kc + 1) * 128], kc == 0, kc == 2)
                mm(Qps[:, mo * 128:(mo + 1) * 128], wblk(WfT, 3, 3 + kc, mo),
                   Vc[:, kc * 128:(kc + 1) * 128], kc == 0, kc == 2)
        for mo in range(3):
            nc.scalar.activation(P_sb[:, mo * 128:(mo + 1) * 128],
                                 Pps[:, mo * 128:(mo + 1) * 128],
                                 Act.Identity, bias=wbias[:, mo:mo + 1],
                                 accum_out=mom[:, mo:mo + 1])
            nc.scalar.activation(Q_sb[:, mo * 128:(mo + 1) * 128],
                                 Qps[:, mo * 128:(mo + 1) * 128], Act.Identity,
                                 accum_out=mom[:, 3 + mo:4 + mo])
            nc.scalar.activation(scrP[:], P_sb[:, mo * 128:(mo + 1) * 128],
                                 Act.Square, accum_out=mom[:, 6 + mo:7 + mo])
            nc.scalar.activation(scrP[:], Q_sb[:, mo * 128:(mo + 1) * 128],
                                 Act.Square, accum_out=mom[:, 9 + mo:10 + mo])
            nc.vector.scalar_tensor_tensor(
                scrP[:], P_sb[:, mo * 128:(mo + 1) * 128], 1.0,
                Q_sb[:, mo * 128:(mo + 1) * 128], op0=Alu.mult, op1=Alu.mult,
                accum_out=mom[:, 12 + mo:13 + mo])
    nc.vector.tensor_copy(mom[:, 15:16], t1col[:])
    nc.vector.tensor_copy(mom[:, 16:17], t2col[:])

    ccf_in = dram.tile([128, 17], f32, name="ccf_in")
    ccf_out = dram.tile([128, 17], f32, name="ccf_out")
    nc.sync.dma_start(ccf_in[:], mom[:])
    nc.gpsimd.collective_compute("AllReduce", Alu.add, replica_groups=RG,
                                 ins=[ccf_in.opt()], outs=[ccf_out.opt()])
    gmom = per.tile([128, 17], f32, name="gmom")
    nc.sync.dma_start(gmom[:], ccf_out[:])

    # T1/T2: partition-sum then broadcast back via K=1 matmul
    t12flat = per.tile([1, 256], f32, name="t12flat")
    nc.sync.dma_start(t12flat[:1].rearrange("1 (p c) -> 1 p c", c=2),
                      gmom[:, 15:17])
    t12 = per.tile([1, 2], f32, name="t12")
    nc.vector.tensor_reduce(t12[:],
                            t12flat.rearrange("1 (p c) -> 1 c p", c=2),
                            axis=AX.X, op=Alu.add)
    t12b_ps = ps_t.tile([128, 2], f32, name="t12b_ps", tag="cps")
    mm(t12b_ps[:], ones1x128[:], t12[:], True, True)
    T1 = per.tile([128, 1], f32, name="T1")
    nc.vector.tensor_copy(T1[:], t12b_ps[:, 0:1])
    T2 = per.tile([128, 1], f32, name="T2")
    nc.vector.tensor_copy(T2[:], t12b_ps[:, 1:2])

    # s = 1 / (std + EPS); var = (T2 - T1^2/N) / (N-1)
    tA = per.tile([128, 1], f32, name="tA")
    nc.vector.tensor_tensor(tA[:], T1[:], T1[:], op=Alu.mult)
    tB = per.tile([128, 1], f32, name="tB")
    nc.vector.scalar_tensor_tensor(tB[:], tA[:], -1.0 / NTOT_DX, T2[:],
                                   op0=Alu.mult, op1=Alu.add)
    stdx = per.tile([128, 1], f32, name="stdx")
    nc.scalar.activation(stdx[:], tB[:], Act.Sqrt,
                         scale=1.0 / (NTOT_DX - 1))
    nc.vector.tensor_scalar(stdx[:], stdx[:], EPS, None, op0=Alu.add)
    s_col = per.tile([128, 1], f32, name="s_col")
    nc.vector.reciprocal(s_col[:], stdx[:])
    s2_col = per.tile([128, 1], f32, name="s2_col")
    nc.vector.tensor_tensor(s2_col[:], s_col[:], s_col[:], op=Alu.mult)

    # BNf (batched over the 3 channel blocks)
    muf = per.tile([128, 3], f32, name="muf")
    nc.vector.scalar_tensor_tensor(muf[:], gmom[:, 3:6], s_col[:],
                                   gmom[:, 0:3], op0=Alu.mult, op1=Alu.add)
    nc.scalar.activation(muf[:], muf[:], Act.Identity, scale=1.0 / NB)
    e2f = per.tile([128, 3], f32, name="e2f")
    nc.vector.scalar_tensor_tensor(e2f[:], gmom[:, 12:15], s_col[:],
                                   gmom[:, 6:9], op0=Alu.mult, op1=Alu.add)
    nc.vector.scalar_tensor_tensor(e2f[:], gmom[:, 12:15], s_col[:],
                                   e2f[:], op0=Alu.mult, op1=Alu.add)
    nc.vector.scalar_tensor_tensor(e2f[:], gmom[:, 9:12], s2_col[:],
                                   e2f[:], op0=Alu.mult, op1=Alu.add)
    nc.scalar.activation(e2f[:], e2f[:], Act.Identity, scale=1.0 / NB)
    varf = per.tile([128, 3], f32, name="varf")
    nc.vector.tensor_tensor(varf[:], muf[:], muf[:], op=Alu.mult)
    nc.vector.tensor_tensor(varf[:], e2f[:], varf[:], op=Alu.subtract)
    scf, shf = bn_scale_shift(varf[:], muf[:], gammaf_sb[:], betaf_sb[:],
                              "bnf", n=3)

    outsb = per.tile([128, C], f32, name="outsb")
    for mo in range(3):
        zf = per.tile([128, 128], f32, name=f"zf_{mo}")
        nc.vector.scalar_tensor_tensor(zf[:],
                                       Q_sb[:, mo * 128:(mo + 1) * 128],
                                       s_col[:],
                                       P_sb[:, mo * 128:(mo + 1) * 128],
                                       op0=Alu.mult, op1=Alu.add)
        fused = per.tile([128, 128], f32, name=f"fused_{mo}")
        nc.scalar.activation(fused[:], zf[:], Act.Relu,
                             bias=shf[:, mo:mo + 1], scale=scf[:, mo:mo + 1])
        pst = ps_t.tile([128, 128], f32, name="wtps", tag="wtps")
        nc.tensor.transpose(pst[:], fused[:], ident[:])
        nc.vector.tensor_copy(outsb[:, mo * 128:(mo + 1) * 128], pst[:])

    nc.sync.dma_start(io["out"].ap(), outsb[:])
    ps_t_cm.__exit__(None, None, None)
    ctx.close()


def _get_built():
    global _BUILT
    if _BUILT is None:
        _BUILT = _build()
    return _BUILT


def make_in_maps(inputs):
    pgf = np.ascontiguousarray(
        np.asarray(inputs["point_groups"], dtype=np.float32).reshape(
            1024, NPTS, 3))
    names = ["W1", "b1", "gamma1", "beta1", "W2", "b2", "W3", "b3",
             "gamma3", "beta3", "W4", "b4", "Wf", "bf", "gammaf", "betaf"]
    base = {n: np.ascontiguousarray(np.asarray(inputs[n], dtype=np.float32))
            for n in names}
    base["alpha"] = np.ascontiguousarray(
        np.asarray(inputs["alpha"], dtype=np.float32).reshape(C))
    base["beta_aff"] = np.ascontiguousarray(
        np.asarray(inputs["beta_aff"], dtype=np.float32).reshape(C))
    base["pg_full"] = pgf
    in_maps = []
    for c in range(NCORES):
        m = dict(base)
        m["pg"] = np.ascontiguousarray(pgf[c * G:(c + 1) * G])
        in_maps.append(m)
    return in_maps


def kernel(**inputs):
    from concourse.bass_utils import run_bass_kernel_spmd

    nc = _get_built()
    in_maps = make_in_maps(inputs)
    res = run_bass_kernel_spmd(nc, in_maps, list(range(NCORES)))
    full = np.concatenate([res.results[c]["out"] for c in range(NCORES)],
                          axis=0)
    return full.reshape(4, 256, C)
